# revision 8
# baseline (speedup 1.0000x reference)
"""Causal self-attention (GQA + RoPE) Trainium2 Bass kernel, 8 NeuronCores.

Sharding: 2-way data parallel over batch x 4-way tensor parallel over heads.
Core c handles batch c//4 and query heads [4*(c%4), 4*(c%4)+4) plus the one
KV head g = c%4 that serves them (n_kv_heads=4 -> no KV replication).
Each core computes a partial [S, D] output (its heads' slice of the out
projection, fp16); the host sums the 4 partials per batch in f32.

Device layouts are transposed ("feature-major"): projections produce qT/kT/vT
[dim, tokens]; attention scores are computed as S^T = kT.T @ qT.  RoPE is
handled by de-interleaving the q/k weight rows on the host so the rotation
pairs become (p, p+64) partition pairs.

The schedule is a chunk pipeline over 512-token query chunks 0..3:
prep(c) = q/kv projections, attn(c), outproj(c), with outproj(c) interleaved
into attn(c+1) so the PE never sits behind the otc normalization chain.
Softmax denominators avoid M=1 PE matmuls: exp tiles (fp16) are accumulated
over key-blocks on the DVE (2x packed mode), then a single ones-vector
matmul per (chunk, head) reduces the accumulator; the reciprocal is
broadcast across partitions on the otherwise-idle Pool engine.
"""

import sys

if "/opt/trn_rl_repo" not in sys.path:
    sys.path.insert(0, "/opt/trn_rl_repo")

import math
from collections import deque

import numpy as np

D_MODEL = 2048
N_HEADS = 16
N_KV_HEADS = 4
ROPE_THETA = 10000.0
B, S = 2, 2048
DK = D_MODEL // N_HEADS          # 128
NCORES = 8
NEG = -1e30

_COMPILED = None
_TRACE = False                   # test.py flips this for profiling runs
_LAST_RESULT = None              # BassKernelResults of the last run


def _build():
    import concourse.bacc as bacc
    import concourse.tile as tile
    from concourse import mybir

    f32 = mybir.dt.float32
    bf16 = mybir.dt.bfloat16
    fp16 = mybir.dt.float16

    nc = bacc.Bacc("TRN2", debug=False, target_bir_lowering=False)

    def inp(name, shape, dt):
        return nc.declare_dram_parameter(name, list(shape), dt, isOutput=False).ap()

    x_d = inp("x", [128, 4, 16, 512], bf16)
    wq_d = inp("wq", [128, 4, 16, 128], bf16)      # m-block major
    wkv_d = inp("wkv", [128, 16, 256], bf16)
    wc_d = inp("wc", [128, 4, 2048], fp16)
    cos_d = inp("cos2", [128, S], f32)
    sin_d = inp("ss", [128, S], f32)
    ident_d = inp("ident", [128, 128], fp16)
    maskpat_d = inp("maskpat", [128, 128], fp16)
    onescol_d = inp("onescol", [128, 1], fp16)
    out_d = nc.declare_dram_parameter("out", [S, D_MODEL], fp16, isOutput=True).ap()

    EXP = mybir.ActivationFunctionType.Exp

    with tile.TileContext(nc) as tc:
        with (
            tc.tile_pool(name="consts", bufs=1) as consts,
            tc.tile_pool(name="tmp", bufs=2) as tmpp,
            tc.tile_pool(name="epool", bufs=5) as epool,
            tc.tile_pool(name="accp", bufs=6) as accp,
            tc.tile_pool(name="rsp", bufs=2) as rsp,
            tc.tile_pool(name="bcp", bufs=2) as bcp,
            tc.tile_pool(name="otcp", bufs=2) as otcp,
            tc.tile_pool(name="vchp", bufs=2) as vchp,
            tc.tile_pool(name="osbp", bufs=4) as osbp,
            tc.tile_pool(name="psB2", bufs=2, space="PSUM") as psB2,
            tc.tile_pool(name="psOT", bufs=2, space="PSUM") as psOT,
        ):
            # ---- constants / weights ----
            wq_sb = consts.tile([128, 4, 16, 128], bf16, tag="wq")
            wkv_sb = consts.tile([128, 16, 256], bf16, tag="wkv")
            wc_sb = consts.tile([128, 4, 2048], fp16, tag="wc")
            c2_sb = consts.tile([128, S], f32, tag="cos2")
            ss_sb = consts.tile([128, S], f32, tag="ss")
            ident_sb = consts.tile([128, 128], fp16, tag="ident")
            maskpat_sb = consts.tile([128, 128], fp16, tag="maskpat")
            ones_sb = consts.tile([128, 1], fp16, tag="onescol")
            zeros_sb = consts.tile([128, 512], fp16, tag="zeros")
            kTr_sb = consts.tile([128, S], fp16, tag="kTr")
            v_sb = consts.tile([128, 16, 128], fp16, tag="V")
            xT = consts.tile([128, 4, 16, 512], bf16, tag="xT")
            qTr_sb = consts.tile([128, 4, 4, 512], fp16, tag="qTr")

            # x chunk-major on the sync queue so chunk 0 lands first
            for c in range(4):
                nc.sync.dma_start(out=xT[:, c], in_=x_d[:, c])
            # weights ordered by first use on the scalar queue
            nc.scalar.dma_start(out=wq_sb[:, 0:1], in_=wq_d[:, 0:1])
            nc.scalar.dma_start(out=wq_sb[:, 1:4], in_=wq_d[:, 1:4])
            nc.scalar.dma_start(out=wkv_sb, in_=wkv_d)
            nc.scalar.dma_start(out=c2_sb, in_=cos_d)
            nc.scalar.dma_start(out=ss_sb, in_=sin_d)
            nc.scalar.dma_start(out=ident_sb, in_=ident_d)
            nc.scalar.dma_start(out=maskpat_sb, in_=maskpat_d)
            nc.scalar.dma_start(out=ones_sb, in_=onescol_d)
            nc.scalar.dma_start(out=wc_sb, in_=wc_d)
            nc.vector.memset(zeros_sb, 0)

            pending_sum = [None]     # deferred ones-mm + normalization chain

            def flush_pending():
                if pending_sum[0] is not None:
                    fn = pending_sum[0]
                    pending_sum[0] = None
                    fn()

            def rope(dst, src, c):
                """dst[128,512] (fp16 SBUF) <- rotate(src[128,512] f32 PSUM)."""
                cs = c2_sb[:, c * 512:(c + 1) * 512]
                sn = ss_sb[:, c * 512:(c + 1) * 512]
                t = tmpp.tile([128, 512], f32, tag="ropesin", name="rsin")
                t2 = tmpp.tile([128, 512], f32, tag="ropecos", name="rcos")
                nc.vector.tensor_mul(t[0:64, :], src[64:128, :], sn[0:64, :])
                nc.vector.tensor_mul(t[64:128, :], src[0:64, :], sn[64:128, :])
                nc.vector.tensor_mul(t2, src, cs)
                nc.vector.tensor_add(dst, t2, t)

            def emit_qproj_pair(c, p, flush_after_first=False):
                tq = slice(c * 512, (c + 1) * 512)
                t = psB2.tile([128, 2, 512], f32, tag="b2", name="b2t")
                for half in range(2):
                    m = 2 * p + half
                    for db in range(16):
                        nc.tensor.matmul(
                            t[:, half, :],
                            lhsT=wq_sb[:, m, db, :],
                            rhs=xT[:, c, db, :],
                            start=(db == 0),
                            stop=(db == 15),
                        )
                    if flush_after_first and half == 0:
                        flush_pending()
                for half in range(2):
                    rope(qTr_sb[:, c, 2 * p + half, :], t[:, half, :], c)

            def emit_kv(c):
                tq = slice(c * 512, (c + 1) * 512)
                t = psB2.tile([128, 2, 512], f32, tag="b2", name="b2t")
                for half in range(2):
                    for db in range(16):
                        nc.tensor.matmul(
                            t[:, half, :],
                            lhsT=wkv_sb[:, db, 128 * half:128 * half + 128],
                            rhs=xT[:, c, db, :],
                            start=(db == 0),
                            stop=(db == 15),
                        )
                rope(kTr_sb[:, tq], t[:, 0, :], c)
                vch = vchp.tile([128, 512], fp16, tag="vch", name="vch")
                nc.scalar.copy(out=vch, in_=t[:, 1, :])
                for rr in range(4):
                    nc.sync.dma_start_transpose(
                        out=v_sb[:, 4 * c + rr, :],
                        in_=vch[:, rr * 128:(rr + 1) * 128],
                    )

            def prep(c, flush=False):
                emit_qproj_pair(c, 0, flush_after_first=flush)
                emit_qproj_pair(c, 1)
                emit_kv(c)

            def prep_fillers(c):
                return [lambda: emit_qproj_pair(c, 0),
                        lambda: emit_qproj_pair(c, 1),
                        lambda: emit_kv(c)]

            fillq = deque()          # PE filler closures (outproj / prep groups)

            def make_po_group(c, j):
                tb, och = j // 2, j % 2

                def emit(c=c, tb=tb, och=och, j=j):
                    po = psB2.tile([128, 2, 512], f32, tag="b2", name="po")
                    for half in range(2):
                        oc = och * 2 + half
                        for h in range(4):
                            nc.tensor.matmul(
                                po[:, half, :],
                                lhsT=otcs[c][:, h, tb * 128:(tb + 1) * 128],
                                rhs=wc_sb[:, h, oc * 512:(oc + 1) * 512],
                                start=(h == 0),
                                stop=(h == 3),
                            )
                    osb = osbp.tile([128, 2, 512], fp16, tag="osb", name="osb")
                    if j % 2 == 0:
                        nc.scalar.copy(out=osb, in_=po)
                    else:
                        nc.vector.tensor_copy(out=osb, in_=po)
                    row = c * 512 + tb * 128
                    nc.sync.dma_start(
                        out=out_d[row:row + 128, och * 1024:(och + 1) * 1024],
                        in_=osb,
                    )
                return emit

            otcs = {}

            def attn_head(c, h, immediate=False, budget=3):
                nkb = 4 * c + 4
                qv = qTr_sb[:, c, h, :]
                ot = psOT.tile([128, 2, 512], f32, tag="ot", name="ott")
                acc0 = accp.tile([128, 512], fp16, tag="acc", name="acc0")
                acc1 = accp.tile([128, 512], fp16, tag="acc", name="acc1") if c > 0 else None

                # unit list: full kb pairs below the diagonal group, then
                # diagonal-block pairs (with per-half causal col ranges)
                units = []
                for j in range(2 * c):
                    units.append(("full", (2 * j, 2 * j + 1)))
                units.append(("diag", (4 * c, 4 * c + 1)))
                units.append(("diag", (4 * c + 2, 4 * c + 3)))
                nu = len(units)

                def emit_unit(i):
                    kind, kbs = units[i]
                    t = psB2.tile([128, 2, 512], f32, tag="b2", name="b2t")
                    for half, kb in enumerate(kbs):
                        col0 = 0 if kind == "full" else 128 * (kb - 4 * c)
                        nc.tensor.matmul(
                            t[:, half, col0:512],
                            lhsT=kTr_sb[:, kb * 128:(kb + 1) * 128],
                            rhs=qv[:, col0:512],
                            start=True,
                            stop=(kind == "full"),
                        )
                        if kind == "diag":
                            # causal mask: accumulate ident.T @ maskpat
                            # (= maskpat) into the diagonal 128-col block
                            nc.tensor.matmul(
                                t[:, half, col0:col0 + 128],
                                lhsT=ident_sb,
                                rhs=maskpat_sb,
                                start=False,
                                stop=True,
                            )
                    return t

                def consume_unit(i, t):
                    kind, kbs = units[i]
                    e = epool.tile([128, 2, 512], fp16, tag="e", name="et")
                    if kind == "full":
                        nc.scalar.activation(out=e, in_=t, func=EXP)
                    else:
                        for half, kb in enumerate(kbs):
                            col0 = 128 * (kb - 4 * c)
                            nc.scalar.activation(
                                out=e[:, half, col0:512],
                                in_=t[:, half, col0:512],
                                func=EXP,
                            )
                    # denominators: accumulate e over key blocks on the DVE
                    for half, kb in enumerate(kbs):
                        col0 = 0 if kind == "full" else 128 * (kb - 4 * c)
                        if c == 0:
                            acc, first = acc0, (kb == 0)
                        else:
                            acc = acc0 if (i % 2 == 0) else acc1
                            first = i < 2 and half == 0
                        if first:
                            nc.vector.tensor_add(
                                acc[:, col0:512],
                                e[:, half, col0:512],
                                zeros_sb[:, col0:512],
                            )
                        else:
                            nc.vector.tensor_add(
                                acc[:, col0:512],
                                acc[:, col0:512],
                                e[:, half, col0:512],
                            )
                    for half, kb in enumerate(kbs):
                        col0 = 0 if kind == "full" else 128 * (kb - 4 * c)
                        nc.tensor.matmul(
                            ot[:, 0, col0:512],
                            lhsT=v_sb[:, kb, :],
                            rhs=e[:, half, col0:512],
                            start=(kb == 0),
                            stop=(kb == nkb - 1),
                        )

                ipoints = {nu // 3, (2 * nu) // 3}
                emitted = {}
                emitted[0] = emit_unit(0)
                if nu > 1:
                    emitted[1] = emit_unit(1)
                flush_pending()
                pops = 0
                if fillq and pops < budget:
                    fillq.popleft()()
                    pops += 1
                for i in range(nu):
                    if i + 2 < nu:
                        emitted[i + 2] = emit_unit(i + 2)
                    consume_unit(i, emitted.pop(i))
                    if i in ipoints and fillq and pops < budget:
                        fillq.popleft()()
                        pops += 1

                if c > 0:
                    nc.gpsimd.tensor_add(acc0, acc0, acc1)

                def emit_sum(ot=ot, acc=acc0, c=c, h=h):
                    nc.tensor.matmul(
                        ot[0:1, 1, :], lhsT=ones_sb, rhs=acc,
                        start=True, stop=True,
                    )
                    rs = rsp.tile([1, 512], f32, tag="rs", name="rs")
                    nc.vector.reciprocal_approx_fast(out=rs, in_=ot[0:1, 1, :])
                    bc = bcp.tile([128, 512], f32, tag="bc", name="bc")
                    nc.gpsimd.partition_broadcast(bc, rs)
                    nc.vector.tensor_mul(otcs[c][:, h, :], ot[:, 0, :], bc)

                if immediate:
                    emit_sum()
                else:
                    pending_sum[0] = emit_sum

            def attn(c, last=False):
                otcs[c] = otcp.tile([128, 4, 512], fp16, tag="otc", name="otct")
                budget = max(1, -(-len(fillq) // 4))
                for h in range(4):
                    attn_head(c, h, immediate=(last and h == 3), budget=budget)

            def drain_fill():
                while fillq:
                    fillq.popleft()()

            with nc.allow_low_precision("fp16 exp-sum accumulators"):
                prep(0)
                prep(1)
                fillq.extend(prep_fillers(2))
                attn(0)
                drain_fill()
                fillq.extend(make_po_group(0, j) for j in range(8))
                fillq.extend(prep_fillers(3))
                attn(1)
                drain_fill()
                fillq.extend(make_po_group(1, j) for j in range(8))
                attn(2)
                drain_fill()
                fillq.extend(make_po_group(2, j) for j in range(8))
                attn(3, last=True)
                drain_fill()
                for j in range(8):
                    make_po_group(3, j)()

    nc.compile()
    return nc


def _host_prep(x, Wq, Wkv, Wc):
    """Shard + relayout the full inputs into the 8 per-core input dicts."""
    import ml_dtypes

    bf = ml_dtypes.bfloat16
    f16 = np.float16
    dk, H, KV = DK, N_HEADS, N_KV_HEADS
    x = np.asarray(x, np.float32)
    Wq = np.asarray(Wq, np.float32)
    Wkv = np.asarray(Wkv, np.float32)
    Wc = np.asarray(Wc, np.float32)

    p = np.concatenate([np.arange(0, dk, 2), np.arange(1, dk, 2)])
    perm_q = np.concatenate([h * dk + p for h in range(H)])
    Wq_p = (Wq / math.sqrt(dk))[perm_q]
    perm_k = np.concatenate([g * dk + p for g in range(KV)])
    Wk_p = Wkv[:KV * dk][perm_k]
    Wv = Wkv[KV * dk:]

    pairs = np.arange(dk // 2, dtype=np.float64)
    freqs = 1.0 / (ROPE_THETA ** (2.0 * pairs / dk))
    ang = np.arange(S, dtype=np.float64)[:, None] * freqs[None, :]
    cos_t = np.cos(ang).astype(np.float32).T  # [64, S]
    sin_t = np.sin(ang).astype(np.float32).T
    c2 = np.ascontiguousarray(np.concatenate([cos_t, cos_t], 0))   # [128, S]
    ss = np.ascontiguousarray(np.concatenate([-sin_t, sin_t], 0))  # [128, S]

    jj = np.arange(128)[None, :]
    pp = np.arange(128)[:, None]
    ident = np.eye(128, dtype=f16)
    maskpat = np.where(pp <= jj, 0.0, -60000.0).astype(f16)
    onescol = np.ones((128, 1), f16)

    maps = []
    for core in range(NCORES):
        b, g = core // 4, core % 4
        wq_l = np.ascontiguousarray(
            Wq_p[512 * g:512 * g + 512].T.reshape(16, 128, 512).transpose(1, 0, 2)
            .reshape(128, 16, 4, 128).transpose(0, 2, 1, 3)
        ).astype(bf)                                   # [128, m, db, 128]
        wkv_sl = np.concatenate(
            [Wk_p[g * dk:(g + 1) * dk], Wv[g * dk:(g + 1) * dk]], 0
        ).T  # [2048, 256]
        wkv_l = np.ascontiguousarray(
            wkv_sl.reshape(16, 128, 256).transpose(1, 0, 2)
        ).astype(bf)
        wc_l = np.ascontiguousarray(
            Wc[:, 512 * g:512 * g + 512].T.reshape(4, 128, 2048).transpose(1, 0, 2)
        ).astype(f16)
        xt_l = np.ascontiguousarray(
            x[b].T.reshape(16, 128, S).transpose(1, 0, 2)
            .reshape(128, 16, 4, 512).transpose(0, 2, 1, 3)
        ).astype(bf)
        maps.append(dict(
            x=xt_l, wq=wq_l, wkv=wkv_l, wc=wc_l,
            cos2=c2, ss=ss, ident=ident, maskpat=maskpat, onescol=onescol,
        ))
    return maps


def kernel(x, Wq, Wkv, Wc):
    global _COMPILED, _LAST_RESULT
    from concourse.bass_utils import run_bass_kernel_spmd

    if _COMPILED is None:
        _COMPILED = _build()
    in_maps = _host_prep(x, Wq, Wkv, Wc)
    res = run_bass_kernel_spmd(
        _COMPILED, in_maps, core_ids=list(range(NCORES)), trace=_TRACE
    )
    _LAST_RESULT = res
    outs = [res.results[i]["out"].astype(np.float32) for i in range(NCORES)]
    full = np.stack(
        [outs[0] + outs[1] + outs[2] + outs[3],
         outs[4] + outs[5] + outs[6] + outs[7]], 0
    )
    return full


# revision 9
# speedup vs baseline: 1.2210x; 1.2210x over previous
"""Causal self-attention (GQA + RoPE) Trainium2 Bass kernel, 8 NeuronCores.

Sharding: 2-way data parallel over batch x 4-way tensor parallel over heads.
Core c handles batch c//4 and query heads [4*(c%4), 4*(c%4)+4) plus the one
KV head g = c%4 that serves them (n_kv_heads=4 -> no KV replication).
Each core computes a partial [S, D] output (its heads' slice of the out
projection, fp16); the host sums the 4 partials per batch in f32.

Device layouts are transposed ("feature-major"): projections produce qT/kT/vT
[dim, tokens]; attention scores are computed as S^T = kT.T @ qT.  RoPE is
handled by de-interleaving the q/k weight rows on the host so the rotation
pairs become (p, p+64) partition pairs.

The schedule is a chunk pipeline over 512-token query chunks 0..3:
prep(c) = q/kv projections, attn(c), outproj(c), with outproj(c) interleaved
into attn(c+1) so the PE never sits behind the otc normalization chain.
Softmax denominators avoid M=1 PE matmuls: exp tiles (fp16) are accumulated
over key-blocks on the DVE (2x packed mode), then a single ones-vector
matmul per (chunk, head) reduces the accumulator; the reciprocal is
broadcast across partitions on the otherwise-idle Pool engine.
"""

import sys

if "/opt/trn_rl_repo" not in sys.path:
    sys.path.insert(0, "/opt/trn_rl_repo")

import math
from collections import deque

import numpy as np

D_MODEL = 2048
N_HEADS = 16
N_KV_HEADS = 4
ROPE_THETA = 10000.0
B, S = 2, 2048
DK = D_MODEL // N_HEADS          # 128
NCORES = 8
NEG = -1e30

_COMPILED = None
_TRACE = False                   # test.py flips this for profiling runs
_LAST_RESULT = None              # BassKernelResults of the last run


def _build():
    import concourse.bacc as bacc
    import concourse.tile as tile
    from concourse import mybir

    f32 = mybir.dt.float32
    bf16 = mybir.dt.bfloat16
    fp16 = mybir.dt.float16

    nc = bacc.Bacc("TRN2", debug=False, target_bir_lowering=False)

    def inp(name, shape, dt):
        return nc.declare_dram_parameter(name, list(shape), dt, isOutput=False).ap()

    x_d = inp("x", [128, 4, 16, 512], bf16)
    wq_d = inp("wq", [128, 4, 16, 128], bf16)      # m-block major
    wkv_d = inp("wkv", [128, 16, 256], bf16)
    wc_d = inp("wc", [128, 4, 2048], fp16)
    cos_d = inp("cos2", [128, S], f32)
    sin_d = inp("ss", [128, S], f32)
    ident_d = inp("ident", [128, 128], fp16)
    maskpat_d = inp("maskpat", [128, 128], fp16)
    onescol_d = inp("onescol", [128, 1], fp16)
    out_d = nc.declare_dram_parameter("out", [S, D_MODEL], fp16, isOutput=True).ap()

    EXP = mybir.ActivationFunctionType.Exp

    with tile.TileContext(nc) as tc:
        with (
            tc.tile_pool(name="consts", bufs=1) as consts,
            tc.tile_pool(name="tmp", bufs=2) as tmpp,
            tc.tile_pool(name="epool", bufs=5) as epool,
            tc.tile_pool(name="accp", bufs=6) as accp,
            tc.tile_pool(name="rsp", bufs=2) as rsp,
            tc.tile_pool(name="bcp", bufs=2) as bcp,
            tc.tile_pool(name="otcp", bufs=2) as otcp,
            tc.tile_pool(name="vchp", bufs=2) as vchp,
            tc.tile_pool(name="osbp", bufs=4) as osbp,
            tc.tile_pool(name="psB2", bufs=2, space="PSUM") as psB2,
            tc.tile_pool(name="psOT", bufs=2, space="PSUM") as psOT,
        ):
            # ---- constants / weights ----
            wq_sb = consts.tile([128, 4, 16, 128], bf16, tag="wq")
            wkv_sb = consts.tile([128, 16, 256], bf16, tag="wkv")
            wc_sb = consts.tile([128, 4, 2048], fp16, tag="wc")
            c2_sb = consts.tile([128, S], f32, tag="cos2")
            ss_sb = consts.tile([128, S], f32, tag="ss")
            ident_sb = consts.tile([128, 128], fp16, tag="ident")
            maskpat_sb = consts.tile([128, 128], fp16, tag="maskpat")
            ones_sb = consts.tile([128, 1], fp16, tag="onescol")
            zeros_sb = consts.tile([128, 512], fp16, tag="zeros")
            kTr_sb = consts.tile([128, S], fp16, tag="kTr")
            v_sb = consts.tile([128, 16, 128], fp16, tag="V")
            xT = consts.tile([128, 4, 16, 512], bf16, tag="xT")
            qTr_sb = consts.tile([128, 4, 4, 512], fp16, tag="qTr")

            # x chunk-major on the sync queue so chunk 0 lands first
            for c in range(4):
                nc.sync.dma_start(out=xT[:, c], in_=x_d[:, c])
            # weights ordered by first use on the scalar queue
            nc.scalar.dma_start(out=wq_sb[:, 0:1], in_=wq_d[:, 0:1])
            nc.scalar.dma_start(out=wq_sb[:, 1:4], in_=wq_d[:, 1:4])
            nc.scalar.dma_start(out=wkv_sb, in_=wkv_d)
            nc.scalar.dma_start(out=c2_sb, in_=cos_d)
            nc.scalar.dma_start(out=ss_sb, in_=sin_d)
            nc.scalar.dma_start(out=ident_sb, in_=ident_d)
            nc.scalar.dma_start(out=maskpat_sb, in_=maskpat_d)
            nc.scalar.dma_start(out=ones_sb, in_=onescol_d)
            nc.scalar.dma_start(out=wc_sb, in_=wc_d)
            nc.vector.memset(zeros_sb, 0)
            # warm the Pool engine's ucode library for partition_broadcast
            # during the initial DMA window (library reloads cost ~6us)
            warm_rs = rsp.tile([1, 512], f32, tag="rs", name="warmrs")
            warm_bc = bcp.tile([128, 512], f32, tag="bc", name="warmbc")
            nc.vector.memset(warm_rs, 0)
            nc.gpsimd.partition_broadcast(warm_bc, warm_rs)

            pending_sum = [None]     # deferred ones-mm + normalization chain

            def flush_pending():
                if pending_sum[0] is not None:
                    fn = pending_sum[0]
                    pending_sum[0] = None
                    fn()

            def rope(dst, src, c):
                """dst[128,512] (fp16 SBUF) <- rotate(src[128,512] f32 PSUM)."""
                cs = c2_sb[:, c * 512:(c + 1) * 512]
                sn = ss_sb[:, c * 512:(c + 1) * 512]
                t = tmpp.tile([128, 512], f32, tag="ropesin", name="rsin")
                t2 = tmpp.tile([128, 512], f32, tag="ropecos", name="rcos")
                nc.vector.tensor_mul(t[0:64, :], src[64:128, :], sn[0:64, :])
                nc.vector.tensor_mul(t[64:128, :], src[0:64, :], sn[64:128, :])
                nc.vector.tensor_mul(t2, src, cs)
                nc.vector.tensor_add(dst, t2, t)

            def emit_qproj_pair(c, p, flush_after_first=False):
                tq = slice(c * 512, (c + 1) * 512)
                t = psB2.tile([128, 2, 512], f32, tag="b2", name="b2t")
                for half in range(2):
                    m = 2 * p + half
                    for db in range(16):
                        nc.tensor.matmul(
                            t[:, half, :],
                            lhsT=wq_sb[:, m, db, :],
                            rhs=xT[:, c, db, :],
                            start=(db == 0),
                            stop=(db == 15),
                        )
                    if flush_after_first and half == 0:
                        flush_pending()
                for half in range(2):
                    rope(qTr_sb[:, c, 2 * p + half, :], t[:, half, :], c)

            def emit_kv(c):
                tq = slice(c * 512, (c + 1) * 512)
                t = psB2.tile([128, 2, 512], f32, tag="b2", name="b2t")
                for half in range(2):
                    for db in range(16):
                        nc.tensor.matmul(
                            t[:, half, :],
                            lhsT=wkv_sb[:, db, 128 * half:128 * half + 128],
                            rhs=xT[:, c, db, :],
                            start=(db == 0),
                            stop=(db == 15),
                        )
                rope(kTr_sb[:, tq], t[:, 0, :], c)
                vch = vchp.tile([128, 512], fp16, tag="vch", name="vch")
                nc.scalar.copy(out=vch, in_=t[:, 1, :])
                for rr in range(4):
                    nc.sync.dma_start_transpose(
                        out=v_sb[:, 4 * c + rr, :],
                        in_=vch[:, rr * 128:(rr + 1) * 128],
                    )

            def prep(c, flush=False):
                emit_qproj_pair(c, 0, flush_after_first=flush)
                emit_qproj_pair(c, 1)
                emit_kv(c)

            def prep_fillers(c):
                return [lambda: emit_qproj_pair(c, 0),
                        lambda: emit_qproj_pair(c, 1),
                        lambda: emit_kv(c)]

            fillq = deque()          # PE filler closures (outproj / prep groups)

            def make_po_group(c, j):
                tb, och = j // 2, j % 2

                def emit(c=c, tb=tb, och=och, j=j):
                    po = psB2.tile([128, 2, 512], f32, tag="b2", name="po")
                    for half in range(2):
                        oc = och * 2 + half
                        for h in range(4):
                            nc.tensor.matmul(
                                po[:, half, :],
                                lhsT=otcs[c][:, h, tb * 128:(tb + 1) * 128],
                                rhs=wc_sb[:, h, oc * 512:(oc + 1) * 512],
                                start=(h == 0),
                                stop=(h == 3),
                            )
                    osb = osbp.tile([128, 2, 512], fp16, tag="osb", name="osb")
                    if j % 2 == 0:
                        nc.scalar.copy(out=osb, in_=po)
                    else:
                        nc.vector.tensor_copy(out=osb, in_=po)
                    row = c * 512 + tb * 128
                    nc.sync.dma_start(
                        out=out_d[row:row + 128, och * 1024:(och + 1) * 1024],
                        in_=osb,
                    )
                return emit

            otcs = {}

            def attn_head(c, h, immediate=False, budget=3):
                nkb = 4 * c + 4
                qv = qTr_sb[:, c, h, :]
                ot = psOT.tile([128, 2, 512], f32, tag="ot", name="ott")
                acc0 = accp.tile([128, 512], fp16, tag="acc", name="acc0")
                acc1 = accp.tile([128, 512], fp16, tag="acc", name="acc1") if c > 0 else None

                # unit list: full kb pairs below the diagonal group, then
                # diagonal-block pairs (with per-half causal col ranges)
                units = []
                for j in range(2 * c):
                    units.append(("full", (2 * j, 2 * j + 1)))
                units.append(("diag", (4 * c, 4 * c + 1)))
                units.append(("diag", (4 * c + 2, 4 * c + 3)))
                nu = len(units)

                def emit_unit(i):
                    kind, kbs = units[i]
                    t = psB2.tile([128, 2, 512], f32, tag="b2", name="b2t")
                    for half, kb in enumerate(kbs):
                        col0 = 0 if kind == "full" else 128 * (kb - 4 * c)
                        nc.tensor.matmul(
                            t[:, half, col0:512],
                            lhsT=kTr_sb[:, kb * 128:(kb + 1) * 128],
                            rhs=qv[:, col0:512],
                            start=True,
                            stop=(kind == "full"),
                        )
                        if kind == "diag":
                            # causal mask: accumulate ident.T @ maskpat
                            # (= maskpat) into the diagonal 128-col block
                            nc.tensor.matmul(
                                t[:, half, col0:col0 + 128],
                                lhsT=ident_sb,
                                rhs=maskpat_sb,
                                start=False,
                                stop=True,
                            )
                    return t

                def consume_unit(i, t):
                    kind, kbs = units[i]
                    e = epool.tile([128, 2, 512], fp16, tag="e", name="et")
                    if kind == "full":
                        nc.scalar.activation(out=e, in_=t, func=EXP)
                    else:
                        for half, kb in enumerate(kbs):
                            col0 = 128 * (kb - 4 * c)
                            nc.scalar.activation(
                                out=e[:, half, col0:512],
                                in_=t[:, half, col0:512],
                                func=EXP,
                            )
                    # denominators: accumulate e over key blocks on the DVE
                    for half, kb in enumerate(kbs):
                        col0 = 0 if kind == "full" else 128 * (kb - 4 * c)
                        if c == 0:
                            acc, first = acc0, (kb == 0)
                        else:
                            acc = acc0 if (i % 2 == 0) else acc1
                            first = i < 2 and half == 0
                        if first:
                            nc.vector.tensor_add(
                                acc[:, col0:512],
                                e[:, half, col0:512],
                                zeros_sb[:, col0:512],
                            )
                        else:
                            nc.vector.tensor_add(
                                acc[:, col0:512],
                                acc[:, col0:512],
                                e[:, half, col0:512],
                            )
                    for half, kb in enumerate(kbs):
                        col0 = 0 if kind == "full" else 128 * (kb - 4 * c)
                        nc.tensor.matmul(
                            ot[:, 0, col0:512],
                            lhsT=v_sb[:, kb, :],
                            rhs=e[:, half, col0:512],
                            start=(kb == 0),
                            stop=(kb == nkb - 1),
                        )

                ipoints = {nu // 3, (2 * nu) // 3}
                emitted = {}
                emitted[0] = emit_unit(0)
                if nu > 1:
                    emitted[1] = emit_unit(1)
                flush_pending()
                pops = 0
                if fillq and pops < budget:
                    fillq.popleft()()
                    pops += 1
                for i in range(nu):
                    if i + 2 < nu:
                        emitted[i + 2] = emit_unit(i + 2)
                    consume_unit(i, emitted.pop(i))
                    if i in ipoints and fillq and pops < budget:
                        fillq.popleft()()
                        pops += 1

                if c > 0:
                    nc.vector.tensor_add(acc0, acc0, acc1)

                def emit_sum(ot=ot, acc=acc0, c=c, h=h):
                    nc.tensor.matmul(
                        ot[0:1, 1, :], lhsT=ones_sb, rhs=acc,
                        start=True, stop=True,
                    )
                    rs = rsp.tile([1, 512], f32, tag="rs", name="rs")
                    nc.vector.reciprocal_approx_fast(out=rs, in_=ot[0:1, 1, :])
                    bc = bcp.tile([128, 512], f32, tag="bc", name="bc")
                    nc.gpsimd.partition_broadcast(bc, rs)
                    nc.vector.tensor_mul(otcs[c][:, h, :], ot[:, 0, :], bc)

                if immediate:
                    emit_sum()
                else:
                    pending_sum[0] = emit_sum

            def attn(c, last=False):
                otcs[c] = otcp.tile([128, 4, 512], fp16, tag="otc", name="otct")
                budget = max(1, -(-len(fillq) // 4))
                for h in range(4):
                    attn_head(c, h, immediate=(last and h == 3), budget=budget)

            def drain_fill():
                while fillq:
                    fillq.popleft()()

            with nc.allow_low_precision("fp16 exp-sum accumulators"):
                prep(0)
                prep(1)
                fillq.extend(prep_fillers(2))
                attn(0)
                drain_fill()
                fillq.extend(make_po_group(0, j) for j in range(8))
                fillq.extend(prep_fillers(3))
                attn(1)
                drain_fill()
                fillq.extend(make_po_group(1, j) for j in range(8))
                attn(2)
                drain_fill()
                fillq.extend(make_po_group(2, j) for j in range(8))
                attn(3, last=True)
                drain_fill()
                for j in range(8):
                    make_po_group(3, j)()

    nc.compile()
    return nc


def _host_prep(x, Wq, Wkv, Wc):
    """Shard + relayout the full inputs into the 8 per-core input dicts."""
    import ml_dtypes

    bf = ml_dtypes.bfloat16
    f16 = np.float16
    dk, H, KV = DK, N_HEADS, N_KV_HEADS
    x = np.asarray(x, np.float32)
    Wq = np.asarray(Wq, np.float32)
    Wkv = np.asarray(Wkv, np.float32)
    Wc = np.asarray(Wc, np.float32)

    p = np.concatenate([np.arange(0, dk, 2), np.arange(1, dk, 2)])
    perm_q = np.concatenate([h * dk + p for h in range(H)])
    Wq_p = (Wq / math.sqrt(dk))[perm_q]
    perm_k = np.concatenate([g * dk + p for g in range(KV)])
    Wk_p = Wkv[:KV * dk][perm_k]
    Wv = Wkv[KV * dk:]

    pairs = np.arange(dk // 2, dtype=np.float64)
    freqs = 1.0 / (ROPE_THETA ** (2.0 * pairs / dk))
    ang = np.arange(S, dtype=np.float64)[:, None] * freqs[None, :]
    cos_t = np.cos(ang).astype(np.float32).T  # [64, S]
    sin_t = np.sin(ang).astype(np.float32).T
    c2 = np.ascontiguousarray(np.concatenate([cos_t, cos_t], 0))   # [128, S]
    ss = np.ascontiguousarray(np.concatenate([-sin_t, sin_t], 0))  # [128, S]

    jj = np.arange(128)[None, :]
    pp = np.arange(128)[:, None]
    ident = np.eye(128, dtype=f16)
    maskpat = np.where(pp <= jj, 0.0, -60000.0).astype(f16)
    onescol = np.ones((128, 1), f16)

    maps = []
    for core in range(NCORES):
        b, g = core // 4, core % 4
        wq_l = np.ascontiguousarray(
            Wq_p[512 * g:512 * g + 512].T.reshape(16, 128, 512).transpose(1, 0, 2)
            .reshape(128, 16, 4, 128).transpose(0, 2, 1, 3)
        ).astype(bf)                                   # [128, m, db, 128]
        wkv_sl = np.concatenate(
            [Wk_p[g * dk:(g + 1) * dk], Wv[g * dk:(g + 1) * dk]], 0
        ).T  # [2048, 256]
        wkv_l = np.ascontiguousarray(
            wkv_sl.reshape(16, 128, 256).transpose(1, 0, 2)
        ).astype(bf)
        wc_l = np.ascontiguousarray(
            Wc[:, 512 * g:512 * g + 512].T.reshape(4, 128, 2048).transpose(1, 0, 2)
        ).astype(f16)
        xt_l = np.ascontiguousarray(
            x[b].T.reshape(16, 128, S).transpose(1, 0, 2)
            .reshape(128, 16, 4, 512).transpose(0, 2, 1, 3)
        ).astype(bf)
        maps.append(dict(
            x=xt_l, wq=wq_l, wkv=wkv_l, wc=wc_l,
            cos2=c2, ss=ss, ident=ident, maskpat=maskpat, onescol=onescol,
        ))
    return maps


def kernel(x, Wq, Wkv, Wc):
    global _COMPILED, _LAST_RESULT
    from concourse.bass_utils import run_bass_kernel_spmd

    if _COMPILED is None:
        _COMPILED = _build()
    in_maps = _host_prep(x, Wq, Wkv, Wc)
    res = run_bass_kernel_spmd(
        _COMPILED, in_maps, core_ids=list(range(NCORES)), trace=_TRACE
    )
    _LAST_RESULT = res
    outs = [res.results[i]["out"].astype(np.float32) for i in range(NCORES)]
    full = np.stack(
        [outs[0] + outs[1] + outs[2] + outs[3],
         outs[4] + outs[5] + outs[6] + outs[7]], 0
    )
    return full


# revision 12
# speedup vs baseline: 1.2646x; 1.0356x over previous
"""Causal self-attention (GQA + RoPE) Trainium2 Bass kernel, 8 NeuronCores.

Sharding: 2-way data parallel over batch x 4-way tensor parallel over heads.
Core c handles batch c//4 and query heads [4*(c%4), 4*(c%4)+4) plus the one
KV head g = c%4 that serves them (n_kv_heads=4 -> no KV replication).
Each core computes a partial [S, D] output (its heads' slice of the out
projection, fp16); the host sums the 4 partials per batch in f32.

Device layouts are transposed ("feature-major"): projections produce qT/kT/vT
[dim, tokens]; attention scores are computed as S^T = kT.T @ qT.  RoPE is
handled by de-interleaving the q/k weight rows on the host so the rotation
pairs become (p, p+64) partition pairs.

The schedule is a chunk pipeline over 512-token query chunks 0..3:
prep(c) = q/kv projections, attn(c), outproj(c), with outproj(c) interleaved
into attn(c+1) so the PE never sits behind the otc normalization chain.
Softmax denominators avoid M=1 PE matmuls: exp tiles (fp16) are accumulated
over key-blocks on the DVE (2x packed mode), then a single ones-vector
matmul per (chunk, head) reduces the accumulator; the reciprocal is
broadcast across partitions on the otherwise-idle Pool engine.
"""

import sys

if "/opt/trn_rl_repo" not in sys.path:
    sys.path.insert(0, "/opt/trn_rl_repo")

import math
from collections import deque

import numpy as np

D_MODEL = 2048
N_HEADS = 16
N_KV_HEADS = 4
ROPE_THETA = 10000.0
B, S = 2, 2048
DK = D_MODEL // N_HEADS          # 128
NCORES = 8
NEG = -1e30

_COMPILED = None
_TRACE = False                   # test.py flips this for profiling runs
_LAST_RESULT = None              # BassKernelResults of the last run


def _build():
    import concourse.bacc as bacc
    import concourse.tile as tile
    from concourse import mybir

    f32 = mybir.dt.float32
    bf16 = mybir.dt.bfloat16
    fp16 = mybir.dt.float16

    nc = bacc.Bacc("TRN2", debug=False, target_bir_lowering=False)

    def inp(name, shape, dt):
        return nc.declare_dram_parameter(name, list(shape), dt, isOutput=False).ap()

    x_d = inp("x", [128, 4, 16, 512], bf16)
    wq_d = inp("wq", [128, 4, 16, 128], bf16)      # m-block major
    wkv_d = inp("wkv", [128, 16, 256], bf16)
    wc_d = inp("wc", [128, 4, 2048], fp16)
    cos_d = inp("cos2", [128, 2, S], fp16)
    sin_d = inp("ss", [128, 2, S], fp16)
    ident_d = inp("ident", [128, 128], fp16)
    maskpat_d = inp("maskpat", [128, 128], fp16)
    onescol_d = inp("onescol", [128, 1], fp16)
    out_d = nc.declare_dram_parameter("out", [S, D_MODEL], fp16, isOutput=True).ap()

    EXP = mybir.ActivationFunctionType.Exp

    with tile.TileContext(nc) as tc:
        with (
            tc.tile_pool(name="consts", bufs=1) as consts,
            tc.tile_pool(name="tmp", bufs=2) as tmpp,
            tc.tile_pool(name="epool", bufs=5) as epool,
            tc.tile_pool(name="accp", bufs=6) as accp,
            tc.tile_pool(name="rsp", bufs=2) as rsp,
            tc.tile_pool(name="bcp", bufs=2) as bcp,
            tc.tile_pool(name="otcp", bufs=2) as otcp,
            tc.tile_pool(name="rcp", bufs=2) as rcpool,
            tc.tile_pool(name="osbp", bufs=4) as osbp,
            tc.tile_pool(name="psB2", bufs=2, space="PSUM") as psB2,
            tc.tile_pool(name="psOT", bufs=2, space="PSUM") as psOT,
        ):
            # ---- constants / weights ----
            wq_sb = consts.tile([128, 4, 16, 128], bf16, tag="wq")
            wkv_sb = consts.tile([128, 16, 256], bf16, tag="wkv")
            wc_sb = consts.tile([128, 4, 2048], fp16, tag="wc")
            c2_sb = consts.tile([128, 2, S], fp16, tag="cos2")
            ss_sb = consts.tile([128, 2, S], fp16, tag="ss")
            ident_sb = consts.tile([128, 128], fp16, tag="ident")
            maskpat_sb = consts.tile([128, 128], fp16, tag="maskpat")
            ones_sb = consts.tile([128, 1], fp16, tag="onescol")
            zeros_sb = consts.tile([128, 512], fp16, tag="zeros")
            kTr_sb = consts.tile([128, S], fp16, tag="kTr")
            v_sb = consts.tile([128, 16, 128], fp16, tag="V")
            xT = consts.tile([128, 4, 16, 512], bf16, tag="xT")
            qTr_sb = consts.tile([128, 4, 4, 512], fp16, tag="qTr")

            # x chunk-major on the sync queue so chunk 0 lands first
            for c in range(4):
                nc.sync.dma_start(out=xT[:, c], in_=x_d[:, c])
            # weights ordered by first use on the scalar queue
            nc.scalar.dma_start(out=wq_sb[:, 0:1], in_=wq_d[:, 0:1])
            nc.scalar.dma_start(out=wq_sb[:, 1:4], in_=wq_d[:, 1:4])
            nc.scalar.dma_start(out=wkv_sb, in_=wkv_d)
            nc.scalar.dma_start(out=c2_sb, in_=cos_d)
            nc.scalar.dma_start(out=ss_sb, in_=sin_d)
            nc.scalar.dma_start(out=ident_sb, in_=ident_d)
            nc.scalar.dma_start(out=maskpat_sb, in_=maskpat_d)
            nc.scalar.dma_start(out=ones_sb, in_=onescol_d)
            nc.scalar.dma_start(out=wc_sb, in_=wc_d)
            nc.vector.memset(zeros_sb, 0)
            # warm the Pool engine's ucode library for partition_broadcast
            # during the initial DMA window (library reloads cost ~6us)
            warm_rs = rsp.tile([1, 512], f32, tag="rs", name="warmrs")
            warm_bc = bcp.tile([128, 512], f32, tag="bc", name="warmbc")
            nc.vector.memset(warm_rs, 0)
            nc.gpsimd.partition_broadcast(warm_bc, warm_rs)

            pending_sum = [None]     # deferred ones-mm + normalization chain

            def flush_pending():
                if pending_sum[0] is not None:
                    fn = pending_sum[0]
                    pending_sum[0] = None
                    fn()

            def rope_pair(dst2, t, c, halves=2):
                """dst2 [128,halves,512] fp16 SBUF <- rotate(t [128,halves,512]
                f32 PSUM).  Swap-muls read PSUM with a 64-partition offset
                (legal for PSUM inputs); the final add runs all-fp16 packed.
                """
                tq = slice(c * 512, (c + 1) * 512)
                cs = c2_sb[:, 0:halves, tq]
                sn = ss_sb[:, 0:halves, tq]
                tt = tmpp.tile([128, 2, 512], fp16, tag="ropesin", name="rsin")
                tt2 = tmpp.tile([128, 2, 512], fp16, tag="ropecos", name="rcos")
                nc.vector.tensor_mul(
                    tt[0:64, 0:halves, :], t[64:128, 0:halves, :], sn[0:64])
                nc.vector.tensor_mul(
                    tt[64:128, 0:halves, :], t[0:64, 0:halves, :], sn[64:128])
                nc.vector.tensor_mul(tt2[:, 0:halves, :], t[:, 0:halves, :], cs)
                nc.vector.tensor_add(
                    dst2, tt2[:, 0:halves, :], tt[:, 0:halves, :])

            def qpair_fillers(c, p):
                st = {}

                def fa():
                    t = psB2.tile([128, 2, 512], f32, tag="b2", name="b2t")
                    st["t"] = t
                    for db in range(16):
                        nc.tensor.matmul(
                            t[:, 0, :],
                            lhsT=wq_sb[:, 2 * p, db, :],
                            rhs=xT[:, c, db, :],
                            start=(db == 0),
                            stop=(db == 15),
                        )

                def fb():
                    t = st["t"]
                    for db in range(16):
                        nc.tensor.matmul(
                            t[:, 1, :],
                            lhsT=wq_sb[:, 2 * p + 1, db, :],
                            rhs=xT[:, c, db, :],
                            start=(db == 0),
                            stop=(db == 15),
                        )
                    rope_pair(qTr_sb[:, c, 2 * p:2 * p + 2, :], t, c)
                return [fa, fb]

            def kv_fillers(c):
                tq = slice(c * 512, (c + 1) * 512)
                st = {}

                def fa():
                    t = psB2.tile([128, 2, 512], f32, tag="b2", name="b2t")
                    st["t"] = t
                    for db in range(16):
                        nc.tensor.matmul(
                            t[:, 0, :],
                            lhsT=wkv_sb[:, db, 0:128],
                            rhs=xT[:, c, db, :],
                            start=(db == 0),
                            stop=(db == 15),
                        )

                def fb():
                    t = st["t"]
                    for db in range(16):
                        nc.tensor.matmul(
                            t[:, 1, :],
                            lhsT=wkv_sb[:, db, 128:256],
                            rhs=xT[:, c, db, :],
                            start=(db == 0),
                            stop=(db == 15),
                        )
                    rope_pair(kTr_sb[:, tq], t, c, halves=1)
                    vch = rcpool.tile([128, 512], fp16, tag="rcp", name="vch")
                    nc.scalar.copy(out=vch, in_=t[:, 1, :])
                    for rr in range(4):
                        nc.sync.dma_start_transpose(
                            out=v_sb[:, 4 * c + rr, :],
                            in_=vch[:, rr * 128:(rr + 1) * 128],
                        )
                return [fa, fb]

            def prep_fillers(c):
                return (qpair_fillers(c, 0) + qpair_fillers(c, 1)
                        + kv_fillers(c))

            def prep(c):
                for f in prep_fillers(c):
                    f()

            fillq = deque()          # PE filler closures (outproj / prep groups)

            def po_fillers(c, j):
                tb, och = j // 2, j % 2
                st = {}

                def mm(po, half):
                    oc = och * 2 + half
                    for h in range(4):
                        nc.tensor.matmul(
                            po[:, half, :],
                            lhsT=otcs[c][:, h, tb * 128:(tb + 1) * 128],
                            rhs=wc_sb[:, h, oc * 512:(oc + 1) * 512],
                            start=(h == 0),
                            stop=(h == 3),
                        )

                def fa():
                    po = psB2.tile([128, 2, 512], f32, tag="b2", name="po")
                    st["po"] = po
                    mm(po, 0)

                def fb():
                    po = st["po"]
                    mm(po, 1)
                    osb = osbp.tile([128, 2, 512], fp16, tag="osb", name="osb")
                    if j % 2 == 0:
                        nc.scalar.copy(out=osb, in_=po)
                    else:
                        nc.vector.tensor_copy(out=osb, in_=po)
                    row = c * 512 + tb * 128
                    nc.sync.dma_start(
                        out=out_d[row:row + 128, och * 1024:(och + 1) * 1024],
                        in_=osb,
                    )
                return [fa, fb]

            otcs = {}

            def attn_head(c, h, immediate=False, budget=3):
                nkb = 4 * c + 4
                qv = qTr_sb[:, c, h, :]
                ot = psOT.tile([128, 2, 512], f32, tag="ot", name="ott")
                acc0 = accp.tile([128, 512], fp16, tag="acc", name="acc0")
                acc1 = accp.tile([128, 512], fp16, tag="acc", name="acc1") if c > 0 else None

                # unit list: full kb pairs below the diagonal group, then
                # diagonal-block pairs (with per-half causal col ranges)
                units = []
                for j in range(2 * c):
                    units.append(("full", (2 * j, 2 * j + 1)))
                units.append(("diag", (4 * c, 4 * c + 1)))
                units.append(("diag", (4 * c + 2, 4 * c + 3)))
                nu = len(units)

                def emit_unit(i):
                    kind, kbs = units[i]
                    t = psB2.tile([128, 2, 512], f32, tag="b2", name="b2t")
                    for half, kb in enumerate(kbs):
                        col0 = 0 if kind == "full" else 128 * (kb - 4 * c)
                        nc.tensor.matmul(
                            t[:, half, col0:512],
                            lhsT=kTr_sb[:, kb * 128:(kb + 1) * 128],
                            rhs=qv[:, col0:512],
                            start=True,
                            stop=(kind == "full"),
                        )
                        if kind == "diag":
                            # causal mask: accumulate ident.T @ maskpat
                            # (= maskpat) into the diagonal 128-col block
                            nc.tensor.matmul(
                                t[:, half, col0:col0 + 128],
                                lhsT=ident_sb,
                                rhs=maskpat_sb,
                                start=False,
                                stop=True,
                            )
                    return t

                def consume_unit(i, t):
                    kind, kbs = units[i]
                    e = epool.tile([128, 2, 512], fp16, tag="e", name="et")
                    if kind == "full":
                        nc.scalar.activation(out=e, in_=t, func=EXP)
                    else:
                        for half, kb in enumerate(kbs):
                            col0 = 128 * (kb - 4 * c)
                            nc.scalar.activation(
                                out=e[:, half, col0:512],
                                in_=t[:, half, col0:512],
                                func=EXP,
                            )
                    # denominators: accumulate e over key blocks on the DVE
                    for half, kb in enumerate(kbs):
                        col0 = 0 if kind == "full" else 128 * (kb - 4 * c)
                        if c == 0:
                            acc, first = acc0, (kb == 0)
                        else:
                            acc = acc0 if (i % 2 == 0) else acc1
                            first = i < 2 and half == 0
                        if first:
                            nc.vector.tensor_add(
                                acc[:, col0:512],
                                e[:, half, col0:512],
                                zeros_sb[:, col0:512],
                            )
                        else:
                            nc.vector.tensor_add(
                                acc[:, col0:512],
                                acc[:, col0:512],
                                e[:, half, col0:512],
                            )
                    for half, kb in enumerate(kbs):
                        col0 = 0 if kind == "full" else 128 * (kb - 4 * c)
                        nc.tensor.matmul(
                            ot[:, 0, col0:512],
                            lhsT=v_sb[:, kb, :],
                            rhs=e[:, half, col0:512],
                            start=(kb == 0),
                            stop=(kb == nkb - 1),
                        )

                emitted = {}
                emitted[0] = emit_unit(0)
                if nu > 1:
                    emitted[1] = emit_unit(1)
                flush_pending()
                pops = 0
                if fillq and pops < budget:
                    fillq.popleft()()
                    pops += 1
                for i in range(nu):
                    if i + 2 < nu:
                        emitted[i + 2] = emit_unit(i + 2)
                    consume_unit(i, emitted.pop(i))
                    if fillq and pops < budget:
                        fillq.popleft()()
                        pops += 1

                if c > 0:
                    nc.vector.tensor_add(acc0, acc0, acc1)

                def emit_sum(ot=ot, acc=acc0, c=c, h=h):
                    nc.tensor.matmul(
                        ot[0:1, 1, :], lhsT=ones_sb, rhs=acc,
                        start=True, stop=True,
                    )
                    rs = rsp.tile([1, 512], f32, tag="rs", name="rs")
                    nc.vector.reciprocal_approx_fast(out=rs, in_=ot[0:1, 1, :])
                    bc = bcp.tile([128, 512], f32, tag="bc", name="bc")
                    nc.gpsimd.partition_broadcast(bc, rs)
                    nc.vector.tensor_mul(otcs[c][:, h, :], ot[:, 0, :], bc)

                if immediate:
                    emit_sum()
                else:
                    pending_sum[0] = emit_sum

            def attn(c, last=False):
                otcs[c] = otcp.tile([128, 4, 512], fp16, tag="otc", name="otct")
                budget = max(1, -(-len(fillq) // 4))
                for h in range(4):
                    attn_head(c, h, immediate=(last and h == 3), budget=budget)

            def drain_fill():
                while fillq:
                    fillq.popleft()()

            with nc.allow_low_precision("fp16 exp-sum accumulators"):
                prep(0)
                prep(1)
                fillq.extend(prep_fillers(2))
                attn(0)
                drain_fill()
                for j in range(8):
                    fillq.extend(po_fillers(0, j))
                fillq.extend(prep_fillers(3))
                attn(1)
                drain_fill()
                for j in range(8):
                    fillq.extend(po_fillers(1, j))
                attn(2)
                drain_fill()
                for j in range(8):
                    fillq.extend(po_fillers(2, j))
                attn(3, last=True)
                drain_fill()
                for j in range(8):
                    for f in po_fillers(3, j):
                        f()

    nc.compile()
    return nc


def _host_prep(x, Wq, Wkv, Wc):
    """Shard + relayout the full inputs into the 8 per-core input dicts."""
    import ml_dtypes

    bf = ml_dtypes.bfloat16
    f16 = np.float16
    dk, H, KV = DK, N_HEADS, N_KV_HEADS
    x = np.asarray(x, np.float32)
    Wq = np.asarray(Wq, np.float32)
    Wkv = np.asarray(Wkv, np.float32)
    Wc = np.asarray(Wc, np.float32)

    p = np.concatenate([np.arange(0, dk, 2), np.arange(1, dk, 2)])
    perm_q = np.concatenate([h * dk + p for h in range(H)])
    Wq_p = (Wq / math.sqrt(dk))[perm_q]
    perm_k = np.concatenate([g * dk + p for g in range(KV)])
    Wk_p = Wkv[:KV * dk][perm_k]
    Wv = Wkv[KV * dk:]

    pairs = np.arange(dk // 2, dtype=np.float64)
    freqs = 1.0 / (ROPE_THETA ** (2.0 * pairs / dk))
    ang = np.arange(S, dtype=np.float64)[:, None] * freqs[None, :]
    cos_t = np.cos(ang).astype(np.float16).T  # [64, S]
    sin_t = np.sin(ang).astype(np.float16).T
    c2_1 = np.concatenate([cos_t, cos_t], 0)                       # [128, S]
    ss_1 = np.concatenate([-sin_t, sin_t], 0)
    c2 = np.ascontiguousarray(np.stack([c2_1, c2_1], 1))           # [128, 2, S]
    ss = np.ascontiguousarray(np.stack([ss_1, ss_1], 1))

    jj = np.arange(128)[None, :]
    pp = np.arange(128)[:, None]
    ident = np.eye(128, dtype=f16)
    maskpat = np.where(pp <= jj, 0.0, -60000.0).astype(f16)
    onescol = np.ones((128, 1), f16)

    maps = []
    for core in range(NCORES):
        b, g = core // 4, core % 4
        wq_l = np.ascontiguousarray(
            Wq_p[512 * g:512 * g + 512].T.reshape(16, 128, 512).transpose(1, 0, 2)
            .reshape(128, 16, 4, 128).transpose(0, 2, 1, 3)
        ).astype(bf)                                   # [128, m, db, 128]
        wkv_sl = np.concatenate(
            [Wk_p[g * dk:(g + 1) * dk], Wv[g * dk:(g + 1) * dk]], 0
        ).T  # [2048, 256]
        wkv_l = np.ascontiguousarray(
            wkv_sl.reshape(16, 128, 256).transpose(1, 0, 2)
        ).astype(bf)
        wc_l = np.ascontiguousarray(
            Wc[:, 512 * g:512 * g + 512].T.reshape(4, 128, 2048).transpose(1, 0, 2)
        ).astype(f16)
        xt_l = np.ascontiguousarray(
            x[b].T.reshape(16, 128, S).transpose(1, 0, 2)
            .reshape(128, 16, 4, 512).transpose(0, 2, 1, 3)
        ).astype(bf)
        maps.append(dict(
            x=xt_l, wq=wq_l, wkv=wkv_l, wc=wc_l,
            cos2=c2, ss=ss, ident=ident, maskpat=maskpat, onescol=onescol,
        ))
    return maps


def kernel(x, Wq, Wkv, Wc):
    global _COMPILED, _LAST_RESULT
    from concourse.bass_utils import run_bass_kernel_spmd

    if _COMPILED is None:
        _COMPILED = _build()
    in_maps = _host_prep(x, Wq, Wkv, Wc)
    res = run_bass_kernel_spmd(
        _COMPILED, in_maps, core_ids=list(range(NCORES)), trace=_TRACE
    )
    _LAST_RESULT = res
    outs = [res.results[i]["out"].astype(np.float32) for i in range(NCORES)]
    full = np.stack(
        [outs[0] + outs[1] + outs[2] + outs[3],
         outs[4] + outs[5] + outs[6] + outs[7]], 0
    )
    return full


# revision 13
# speedup vs baseline: 1.5217x; 1.2033x over previous
"""Causal self-attention (GQA + RoPE) Trainium2 Bass kernel, 8 NeuronCores.

Sharding: 2-way data parallel over batch x 4-way tensor parallel over heads.
Core c handles batch c//4 and query heads [4*(c%4), 4*(c%4)+4) plus the one
KV head g = c%4 that serves them (n_kv_heads=4 -> no KV replication).
Each core computes a partial [S, D] output (its heads' slice of the out
projection, fp16); the host sums the 4 partials per batch in f32.

Device layouts are transposed ("feature-major"): projections produce qT/kT/vT
[dim, tokens]; attention scores are computed as S^T = kT.T @ qT.  RoPE is
handled by de-interleaving the q/k weight rows on the host so the rotation
pairs become (p, p+64) partition pairs.

The schedule is a chunk pipeline over 512-token query chunks 0..3:
prep(c) = q/kv projections, attn(c), outproj(c), with outproj(c) interleaved
into attn(c+1) so the PE never sits behind the otc normalization chain.
Softmax denominators avoid M=1 PE matmuls: exp tiles (fp16) are accumulated
over key-blocks on the DVE (2x packed mode), then a single ones-vector
matmul per (chunk, head) reduces the accumulator; the reciprocal is
broadcast across partitions on the otherwise-idle Pool engine.
"""

import sys

if "/opt/trn_rl_repo" not in sys.path:
    sys.path.insert(0, "/opt/trn_rl_repo")

import math
from collections import deque

import numpy as np

D_MODEL = 2048
N_HEADS = 16
N_KV_HEADS = 4
ROPE_THETA = 10000.0
B, S = 2, 2048
DK = D_MODEL // N_HEADS          # 128
NCORES = 8
NEG = -1e30

_COMPILED = None
_TRACE = False                   # test.py flips this for profiling runs
_LAST_RESULT = None              # BassKernelResults of the last run


def _build():
    import concourse.bacc as bacc
    import concourse.tile as tile
    from concourse import mybir
    from concourse.bass import broadcast_tensor_aps

    f32 = mybir.dt.float32
    bf16 = mybir.dt.bfloat16
    fp16 = mybir.dt.float16

    nc = bacc.Bacc("TRN2", debug=False, target_bir_lowering=False)

    def inp(name, shape, dt):
        return nc.declare_dram_parameter(name, list(shape), dt, isOutput=False).ap()

    x_d = inp("x", [128, 4, 16, 512], bf16)
    wq_d = inp("wq", [128, 4, 16, 128], bf16)      # m-block major
    wkv_d = inp("wkv", [128, 16, 256], bf16)
    wc_d = inp("wc", [128, 4, 2048], fp16)
    cos_d = inp("cos2", [128, 1, S], fp16)
    sin_d = inp("ss", [128, 1, S], fp16)
    ident_d = inp("ident", [128, 128], fp16)
    maskpat_d = inp("maskpat", [128, 128], fp16)
    onescol_d = inp("onescol", [128, 1], fp16)
    out_d = nc.declare_dram_parameter("out", [S, D_MODEL], fp16, isOutput=True).ap()

    EXP = mybir.ActivationFunctionType.Exp

    with tile.TileContext(nc) as tc:
        with (
            tc.tile_pool(name="consts", bufs=1) as consts,
            tc.tile_pool(name="tmp", bufs=2) as tmpp,
            tc.tile_pool(name="epool", bufs=5) as epool,
            tc.tile_pool(name="accp", bufs=6) as accp,
            tc.tile_pool(name="rsp", bufs=2) as rsp,
            tc.tile_pool(name="bcp", bufs=2) as bcp,
            tc.tile_pool(name="otcp", bufs=2) as otcp,
            tc.tile_pool(name="rcp", bufs=2) as rcpool,
            tc.tile_pool(name="osbp", bufs=4) as osbp,
            tc.tile_pool(name="psB2", bufs=2, space="PSUM") as psB2,
            tc.tile_pool(name="psOT", bufs=2, space="PSUM") as psOT,
        ):
            # ---- constants / weights ----
            wq_sb = consts.tile([128, 4, 16, 128], bf16, tag="wq")
            wkv_sb = consts.tile([128, 16, 256], bf16, tag="wkv")
            wc_sb = consts.tile([128, 4, 2048], fp16, tag="wc")
            c2_sb = consts.tile([128, 1, S], fp16, tag="cos2")
            ss_sb = consts.tile([128, 1, S], fp16, tag="ss")
            ident_sb = consts.tile([128, 128], fp16, tag="ident")
            maskpat_sb = consts.tile([128, 128], fp16, tag="maskpat")
            ones_sb = consts.tile([128, 1], fp16, tag="onescol")
            zeros_sb = consts.tile([128, 512], fp16, tag="zeros")
            kTr_sb = consts.tile([128, S], fp16, tag="kTr")
            v_sb = consts.tile([128, 16, 128], fp16, tag="V")
            xT = consts.tile([128, 4, 16, 512], bf16, tag="xT")
            qTr_sb = consts.tile([128, 4, 4, 512], fp16, tag="qTr")

            # x chunk-major on the sync queue; chunk 0 split by db-group so
            # the first projection matmul can start after ~0.5MB
            for g in range(4):
                nc.sync.dma_start(
                    out=xT[:, 0, 4 * g:4 * g + 4], in_=x_d[:, 0, 4 * g:4 * g + 4]
                )
            for c in range(1, 4):
                nc.sync.dma_start(out=xT[:, c], in_=x_d[:, c])
            # weights ordered by first use on the scalar queue
            nc.scalar.dma_start(out=wq_sb[:, 0:1], in_=wq_d[:, 0:1])
            nc.scalar.dma_start(out=c2_sb, in_=cos_d)
            nc.scalar.dma_start(out=ss_sb, in_=sin_d)
            nc.scalar.dma_start(out=wq_sb[:, 1:4], in_=wq_d[:, 1:4])
            nc.scalar.dma_start(out=wkv_sb, in_=wkv_d)
            nc.scalar.dma_start(out=ident_sb, in_=ident_d)
            nc.scalar.dma_start(out=maskpat_sb, in_=maskpat_d)
            nc.scalar.dma_start(out=ones_sb, in_=onescol_d)
            nc.scalar.dma_start(out=wc_sb, in_=wc_d)
            nc.vector.memset(zeros_sb, 0)
            # warm the Pool engine's ucode library for partition_broadcast
            # during the initial DMA window (library reloads cost ~6us)
            warm_rs = rsp.tile([1, 512], f32, tag="rs", name="warmrs")
            warm_bc = bcp.tile([128, 512], f32, tag="bc", name="warmbc")
            nc.vector.memset(warm_rs, 0)
            nc.gpsimd.partition_broadcast(warm_bc, warm_rs)

            pending_sum = [None]     # deferred ones-mm + normalization chain

            def flush_pending():
                if pending_sum[0] is not None:
                    fn = pending_sum[0]
                    pending_sum[0] = None
                    fn()

            def rope_pair(dst2, t, c, halves=2):
                """dst2 [128,halves,512] fp16 SBUF <- rotate(t [128,halves,512]
                f32 PSUM).  Swap-muls read PSUM with a 64-partition offset
                (legal for PSUM inputs); the final add runs all-fp16 packed.
                """
                tq = slice(c * 512, (c + 1) * 512)
                tt = tmpp.tile([128, 2, 512], fp16, tag="ropesin", name="rsin")
                tt2 = tmpp.tile([128, 2, 512], fp16, tag="ropecos", name="rcos")

                def bc(tab, ps, pe, ref):
                    a = tab[ps:pe, 0:1, tq]
                    if halves == 1:
                        return a
                    a2, _ = broadcast_tensor_aps(a, ref)
                    return a2

                t_hi = t[64:128, 0:halves, :]
                t_lo = t[0:64, 0:halves, :]
                t_all = t[:, 0:halves, :]
                nc.vector.tensor_mul(
                    tt[0:64, 0:halves, :], t_hi, bc(ss_sb, 0, 64, t_hi))
                nc.vector.tensor_mul(
                    tt[64:128, 0:halves, :], t_lo, bc(ss_sb, 64, 128, t_lo))
                nc.vector.tensor_mul(
                    tt2[:, 0:halves, :], t_all, bc(c2_sb, 0, 128, t_all))
                nc.vector.tensor_add(
                    dst2, tt2[:, 0:halves, :], tt[:, 0:halves, :])

            def qpair_fillers(c, p):
                st = {}

                def fa():
                    t = psB2.tile([128, 2, 512], f32, tag="b2", name="b2t")
                    st["t"] = t
                    for db in range(16):
                        nc.tensor.matmul(
                            t[:, 0, :],
                            lhsT=wq_sb[:, 2 * p, db, :],
                            rhs=xT[:, c, db, :],
                            start=(db == 0),
                            stop=(db == 15),
                        )

                def fb():
                    t = st["t"]
                    for db in range(16):
                        nc.tensor.matmul(
                            t[:, 1, :],
                            lhsT=wq_sb[:, 2 * p + 1, db, :],
                            rhs=xT[:, c, db, :],
                            start=(db == 0),
                            stop=(db == 15),
                        )
                    rope_pair(qTr_sb[:, c, 2 * p:2 * p + 2, :], t, c)
                return [fa, fb]

            def kv_fillers(c):
                tq = slice(c * 512, (c + 1) * 512)
                st = {}

                def fa():
                    t = psB2.tile([128, 2, 512], f32, tag="b2", name="b2t")
                    st["t"] = t
                    for db in range(16):
                        nc.tensor.matmul(
                            t[:, 0, :],
                            lhsT=wkv_sb[:, db, 0:128],
                            rhs=xT[:, c, db, :],
                            start=(db == 0),
                            stop=(db == 15),
                        )

                def fb():
                    t = st["t"]
                    for db in range(16):
                        nc.tensor.matmul(
                            t[:, 1, :],
                            lhsT=wkv_sb[:, db, 128:256],
                            rhs=xT[:, c, db, :],
                            start=(db == 0),
                            stop=(db == 15),
                        )
                    rope_pair(kTr_sb[:, tq], t, c, halves=1)
                    vch = rcpool.tile([128, 512], fp16, tag="rcp", name="vch")
                    nc.scalar.copy(out=vch, in_=t[:, 1, :])
                    for rr in range(4):
                        nc.sync.dma_start_transpose(
                            out=v_sb[:, 4 * c + rr, :],
                            in_=vch[:, rr * 128:(rr + 1) * 128],
                        )
                return [fa, fb]

            def prep_fillers(c):
                return (qpair_fillers(c, 0) + qpair_fillers(c, 1)
                        + kv_fillers(c))

            def prep(c):
                for f in prep_fillers(c):
                    f()

            fillq = deque()          # PE filler closures (outproj / prep groups)

            def po_fillers(c, j):
                tb, och = j // 2, j % 2
                st = {}

                def mm(po, half):
                    oc = och * 2 + half
                    for h in range(4):
                        nc.tensor.matmul(
                            po[:, half, :],
                            lhsT=otcs[c][:, h, tb * 128:(tb + 1) * 128],
                            rhs=wc_sb[:, h, oc * 512:(oc + 1) * 512],
                            start=(h == 0),
                            stop=(h == 3),
                        )

                def fa():
                    po = psB2.tile([128, 2, 512], f32, tag="b2", name="po")
                    st["po"] = po
                    mm(po, 0)

                def fb():
                    po = st["po"]
                    mm(po, 1)
                    osb = osbp.tile([128, 2, 512], fp16, tag="osb", name="osb")
                    if j % 2 == 0:
                        nc.scalar.copy(out=osb, in_=po)
                    else:
                        nc.vector.tensor_copy(out=osb, in_=po)
                    row = c * 512 + tb * 128
                    nc.sync.dma_start(
                        out=out_d[row:row + 128, och * 1024:(och + 1) * 1024],
                        in_=osb,
                    )
                return [fa, fb]

            otcs = {}

            def attn_head(c, h, immediate=False, budget=3):
                nkb = 4 * c + 4
                qv = qTr_sb[:, c, h, :]
                ot = psOT.tile([128, 2, 512], f32, tag="ot", name="ott")
                acc0 = accp.tile([128, 512], fp16, tag="acc", name="acc0")
                acc1 = accp.tile([128, 512], fp16, tag="acc", name="acc1") if c > 0 else None

                # unit list: full kb pairs below the diagonal group, then
                # diagonal-block pairs (with per-half causal col ranges)
                units = []
                for j in range(2 * c):
                    units.append(("full", (2 * j, 2 * j + 1)))
                units.append(("diag", (4 * c, 4 * c + 1)))
                units.append(("diag", (4 * c + 2, 4 * c + 3)))
                nu = len(units)

                def emit_unit(i):
                    kind, kbs = units[i]
                    t = psB2.tile([128, 2, 512], f32, tag="b2", name="b2t")
                    for half, kb in enumerate(kbs):
                        col0 = 0 if kind == "full" else 128 * (kb - 4 * c)
                        nc.tensor.matmul(
                            t[:, half, col0:512],
                            lhsT=kTr_sb[:, kb * 128:(kb + 1) * 128],
                            rhs=qv[:, col0:512],
                            start=True,
                            stop=(kind == "full"),
                        )
                        if kind == "diag":
                            # causal mask: accumulate ident.T @ maskpat
                            # (= maskpat) into the diagonal 128-col block
                            nc.tensor.matmul(
                                t[:, half, col0:col0 + 128],
                                lhsT=ident_sb,
                                rhs=maskpat_sb,
                                start=False,
                                stop=True,
                            )
                    return t

                def consume_unit(i, t):
                    kind, kbs = units[i]
                    e = epool.tile([128, 2, 512], fp16, tag="e", name="et")
                    if kind == "full":
                        nc.scalar.activation(out=e, in_=t, func=EXP)
                    else:
                        for half, kb in enumerate(kbs):
                            col0 = 128 * (kb - 4 * c)
                            nc.scalar.activation(
                                out=e[:, half, col0:512],
                                in_=t[:, half, col0:512],
                                func=EXP,
                            )
                    # denominators: accumulate e over key blocks on the DVE
                    for half, kb in enumerate(kbs):
                        col0 = 0 if kind == "full" else 128 * (kb - 4 * c)
                        if c == 0:
                            acc, first = acc0, (kb == 0)
                        else:
                            acc = acc0 if (i % 2 == 0) else acc1
                            first = i < 2 and half == 0
                        if first:
                            nc.vector.tensor_add(
                                acc[:, col0:512],
                                e[:, half, col0:512],
                                zeros_sb[:, col0:512],
                            )
                        else:
                            nc.vector.tensor_add(
                                acc[:, col0:512],
                                acc[:, col0:512],
                                e[:, half, col0:512],
                            )
                    for half, kb in enumerate(kbs):
                        col0 = 0 if kind == "full" else 128 * (kb - 4 * c)
                        nc.tensor.matmul(
                            ot[:, 0, col0:512],
                            lhsT=v_sb[:, kb, :],
                            rhs=e[:, half, col0:512],
                            start=(kb == 0),
                            stop=(kb == nkb - 1),
                        )

                emitted = {}
                emitted[0] = emit_unit(0)
                if nu > 1:
                    emitted[1] = emit_unit(1)
                flush_pending()
                pops = 0
                if fillq and pops < budget:
                    fillq.popleft()()
                    pops += 1
                for i in range(nu):
                    if i + 2 < nu:
                        emitted[i + 2] = emit_unit(i + 2)
                    consume_unit(i, emitted.pop(i))
                    if fillq and pops < budget:
                        fillq.popleft()()
                        pops += 1

                if c > 0:
                    nc.vector.tensor_add(acc0, acc0, acc1)

                def emit_sum(ot=ot, acc=acc0, c=c, h=h):
                    nc.tensor.matmul(
                        ot[0:1, 1, :], lhsT=ones_sb, rhs=acc,
                        start=True, stop=True,
                    )
                    rs = rsp.tile([1, 512], f32, tag="rs", name="rs")
                    nc.vector.reciprocal_approx_fast(out=rs, in_=ot[0:1, 1, :])
                    bc = bcp.tile([128, 512], f32, tag="bc", name="bc")
                    nc.gpsimd.partition_broadcast(bc, rs)
                    nc.vector.tensor_mul(otcs[c][:, h, :], ot[:, 0, :], bc)

                if immediate:
                    emit_sum()
                else:
                    pending_sum[0] = emit_sum

            def attn(c, last=False):
                otcs[c] = otcp.tile([128, 4, 512], fp16, tag="otc", name="otct")
                budget = max(1, -(-len(fillq) // 4))
                for h in range(4):
                    attn_head(c, h, immediate=(last and h == 3), budget=budget)

            def drain_fill():
                while fillq:
                    fillq.popleft()()

            with nc.allow_low_precision("fp16 exp-sum accumulators"):
                prep(0)
                prep(1)
                fillq.extend(prep_fillers(2))
                attn(0)
                drain_fill()
                for j in range(8):
                    fillq.extend(po_fillers(0, j))
                fillq.extend(prep_fillers(3))
                attn(1)
                drain_fill()
                for j in range(8):
                    fillq.extend(po_fillers(1, j))
                attn(2)
                drain_fill()
                for j in range(8):
                    fillq.extend(po_fillers(2, j))
                attn(3, last=True)
                drain_fill()
                for j in range(8):
                    for f in po_fillers(3, j):
                        f()

    nc.compile()
    return nc


def _host_prep(x, Wq, Wkv, Wc):
    """Shard + relayout the full inputs into the 8 per-core input dicts."""
    import ml_dtypes

    bf = ml_dtypes.bfloat16
    f16 = np.float16
    dk, H, KV = DK, N_HEADS, N_KV_HEADS
    x = np.asarray(x, np.float32)
    Wq = np.asarray(Wq, np.float32)
    Wkv = np.asarray(Wkv, np.float32)
    Wc = np.asarray(Wc, np.float32)

    p = np.concatenate([np.arange(0, dk, 2), np.arange(1, dk, 2)])
    perm_q = np.concatenate([h * dk + p for h in range(H)])
    Wq_p = (Wq / math.sqrt(dk))[perm_q]
    perm_k = np.concatenate([g * dk + p for g in range(KV)])
    Wk_p = Wkv[:KV * dk][perm_k]
    Wv = Wkv[KV * dk:]

    pairs = np.arange(dk // 2, dtype=np.float64)
    freqs = 1.0 / (ROPE_THETA ** (2.0 * pairs / dk))
    ang = np.arange(S, dtype=np.float64)[:, None] * freqs[None, :]
    cos_t = np.cos(ang).astype(np.float16).T  # [64, S]
    sin_t = np.sin(ang).astype(np.float16).T
    c2 = np.ascontiguousarray(
        np.concatenate([cos_t, cos_t], 0)[:, None, :])             # [128, 1, S]
    ss = np.ascontiguousarray(
        np.concatenate([-sin_t, sin_t], 0)[:, None, :])

    jj = np.arange(128)[None, :]
    pp = np.arange(128)[:, None]
    ident = np.eye(128, dtype=f16)
    maskpat = np.where(pp <= jj, 0.0, -60000.0).astype(f16)
    onescol = np.ones((128, 1), f16)

    maps = []
    for core in range(NCORES):
        b, g = core // 4, core % 4
        wq_l = np.ascontiguousarray(
            Wq_p[512 * g:512 * g + 512].T.reshape(16, 128, 512).transpose(1, 0, 2)
            .reshape(128, 16, 4, 128).transpose(0, 2, 1, 3)
        ).astype(bf)                                   # [128, m, db, 128]
        wkv_sl = np.concatenate(
            [Wk_p[g * dk:(g + 1) * dk], Wv[g * dk:(g + 1) * dk]], 0
        ).T  # [2048, 256]
        wkv_l = np.ascontiguousarray(
            wkv_sl.reshape(16, 128, 256).transpose(1, 0, 2)
        ).astype(bf)
        wc_l = np.ascontiguousarray(
            Wc[:, 512 * g:512 * g + 512].T.reshape(4, 128, 2048).transpose(1, 0, 2)
        ).astype(f16)
        xt_l = np.ascontiguousarray(
            x[b].T.reshape(16, 128, S).transpose(1, 0, 2)
            .reshape(128, 16, 4, 512).transpose(0, 2, 1, 3)
        ).astype(bf)
        maps.append(dict(
            x=xt_l, wq=wq_l, wkv=wkv_l, wc=wc_l,
            cos2=c2, ss=ss, ident=ident, maskpat=maskpat, onescol=onescol,
        ))
    return maps


def kernel(x, Wq, Wkv, Wc):
    global _COMPILED, _LAST_RESULT
    from concourse.bass_utils import run_bass_kernel_spmd

    if _COMPILED is None:
        _COMPILED = _build()
    in_maps = _host_prep(x, Wq, Wkv, Wc)
    res = run_bass_kernel_spmd(
        _COMPILED, in_maps, core_ids=list(range(NCORES)), trace=_TRACE
    )
    _LAST_RESULT = res
    outs = [res.results[i]["out"].astype(np.float32) for i in range(NCORES)]
    full = np.stack(
        [outs[0] + outs[1] + outs[2] + outs[3],
         outs[4] + outs[5] + outs[6] + outs[7]], 0
    )
    return full
